# revision 1
# baseline (speedup 1.0000x reference)
"""Trainium2 Bass kernel for nn_Attention (dense transformer MHA block).

Reference computation (fp32):
    qkv = x @ w_qkv.T            # [B,N,3C]
    q,k,v per head; scores = q k^T / sqrt(D); attn = softmax(scores)
    o = attn @ v;  y = o @ w_proj.T + b_proj

Sharding over 8 NeuronCores (data-parallel over batch x tensor-parallel over
heads): core c -> (batch b = c//4, head group g = c%4, heads 4g..4g+3).
Each core computes q/k/v for its 4 heads over the full 2048-token sequence,
runs attention locally, and multiplies by its row-slice of w_proj, producing
a PARTIAL output [2048, 1024].  The 4 partials per batch are summed on the
host (numpy) together with the bias — no device collectives.

All matmuls run in bf16 with fp32 PSUM accumulation.  Scores are computed
transposed ([kv, q]) so exp(scores^T) feeds the A@V matmul directly; V gets an
extra ones-column so the same matmul accumulates the softmax denominator
(row 64 of the PSUM accumulator).  Softmax skips the max-subtraction (logits
are ~N(0,1); exp is safe in fp32), which is mathematically identical.
"""

import numpy as np

B, N, C = 2, 2048, 1024
H, D = 16, 64
NCORES = 8
GROUPS = 4              # head groups (tensor-parallel)
HG = H // GROUPS        # 4 heads per core
CG = HG * D             # 256 channels per core
P = 128
KT = C // P             # 8 contraction subtiles for C=1024
KV_CHUNKS = N // P      # 16 key/value chunks of 128 rows
QT = N // 512           # 4 query tiles of 512
VB = D + 1              # v block width incl. ones column (65)
SCALE = 1.0 / float(np.sqrt(D))

_CACHED_NC = None


def _build_nc():
    from contextlib import ExitStack

    import concourse.bass as bass
    import concourse.mybir as mybir
    import concourse.tile as tile
    from concourse import bacc

    f32 = mybir.dt.float32
    bf16 = mybir.dt.bfloat16
    AF = mybir.ActivationFunctionType

    nc = bacc.Bacc("TRN2", target_bir_lowering=False, debug=False,
                   num_devices=NCORES)

    # per-core inputs (host pre-sharded / pre-transposed)
    xT = nc.dram_tensor("xT", [C, N], bf16, kind="ExternalInput")
    wqkT = nc.dram_tensor("wqkT", [C, 2 * CG], bf16, kind="ExternalInput")
    wvT = nc.dram_tensor("wvT", [C, CG], bf16, kind="ExternalInput")
    wpT = nc.dram_tensor("wpT", [CG, C], bf16, kind="ExternalInput")
    f16 = mybir.dt.float16
    yp = nc.dram_tensor("yp", [N, C], f16, kind="ExternalOutput")

    with tile.TileContext(nc) as tc:
        with ExitStack() as ctx:
            singles = ctx.enter_context(tc.tile_pool(name="singles", bufs=1))
            tmp = ctx.enter_context(tc.tile_pool(name="tmp", bufs=3))
            ps_big = ctx.enter_context(
                tc.tile_pool(name="ps_big", bufs=3, space="PSUM"))
            ps1 = ctx.enter_context(
                tc.tile_pool(name="ps1", bufs=2, space="PSUM"))
            dscratch = ctx.enter_context(
                tc.tile_pool(name="dscratch", bufs=2, space="DRAM"))

            # ---- persistent SBUF tensors -------------------------------
            xT_sb = singles.tile([P, KT, N], bf16)         # x^T (c on part)
            wqk_sb = singles.tile([P, KT, 2 * CG], bf16)   # q|k weight cols
            wv_sb = singles.tile([P, KT, CG], bf16)
            wp_sb = singles.tile([P, CG // P, C], bf16)
            qT_sb = singles.tile([P, HG // 2, N], bf16)    # q^T (d on part)
            kT_sb = singles.tile([P, HG // 2, N], bf16)    # k^T (d on part)
            v_sb = singles.tile([P, KV_CHUNKS, HG * VB], bf16)
            oT_sb = singles.tile([P, CG // P, N], bf16)    # normalized o^T
            ones_bf = singles.tile([1, D], bf16)

            # ---- load inputs ------------------------------------------
            xT_ap = xT.ap().rearrange("(g p) r -> p g r", p=P)
            wqk_ap = wqkT.ap().rearrange("(g p) o -> p g o", p=P)
            # first matmul needs wqk + xT chunk 0 — issue those first, on
            # separate DMA queues so the loads run in parallel
            for j in range(KT):
                nc.scalar.dma_start(wqk_sb[:, j, :], wqk_ap[:, j, :])
            for j in range(KT):
                nc.sync.dma_start(xT_sb[:, j, 0:512], xT_ap[:, j, 0:512])
            for nchunk in range(1, QT):
                nc.sync.dma_start(
                    xT_sb[:, :, nchunk * 512:(nchunk + 1) * 512],
                    xT_ap[:, :, nchunk * 512:(nchunk + 1) * 512])
            nc.scalar.dma_start(
                wv_sb[:], wvT.ap().rearrange("(g p) o -> p g o", p=P))
            nc.scalar.dma_start(
                wp_sb[:], wpT.ap().rearrange("(g p) o -> p g o", p=P))
            # whole-tile memset to 1.0; the v copies below overwrite the data
            # columns, leaving the per-head ones columns for the denominator
            nc.vector.memset(v_sb[:], 1.0)
            nc.vector.memset(ones_bf[:], 1.0)
            v_view = v_sb[:].rearrange("p c (h e) -> p c h e", e=VB)

            # ---- q^T / k^T / v projections -----------------------------
            # wqk columns: 0..CG-1 = q channels, CG..2CG-1 = k channels
            # j outer so one LDWEIGHTS feeds 4 matmuls (one per query chunk);
            # the 4 output tiles accumulate in parallel PSUM banks.
            def qk_mtile(m):
                dst = qT_sb if m < CG // P else kT_sb
                dm = m % (CG // P)
                pts = [ps_big.tile([P, 1024], f32, tag="sc",
                                   name=f"pts{m}_{i}")
                       for i in range(QT // 2)]
                for j in range(KT):
                    for nchunk in range(QT):
                        nc.tensor.matmul(
                            pts[nchunk // 2][:, (nchunk % 2) * 512:
                                             (nchunk % 2) * 512 + 512],
                            wqk_sb[:, j, m * P:(m + 1) * P],
                            xT_sb[:, j, nchunk * 512:(nchunk + 1) * 512],
                            start=(j == 0), stop=(j == KT - 1))
                for nchunk in range(QT):
                    nc.vector.tensor_copy(
                        out=dst[:, dm, nchunk * 512:(nchunk + 1) * 512],
                        in_=pts[nchunk // 2][:, (nchunk % 2) * 512:
                                             (nchunk % 2) * 512 + 512])

            def v_rtile(rt):
                pt = ps_big.tile([P, 1024], f32, tag="sc")
                for j in range(KT):
                    nc.tensor.matmul(
                        pt[:, :CG], xT_sb[:, j, rt * P:(rt + 1) * P],
                        wv_sb[:, j, :], start=(j == 0), stop=(j == KT - 1))
                nc.vector.tensor_copy(
                    out=v_view[:, rt, :, :D],
                    in_=pt[:, :CG].rearrange("p (h d) -> p h d", d=D))

            # emission order minimizes the PE lead-in before the first
            # score matmuls: k/q of pair 0 first, then v (needed by the
            # first A@V ~20us later), then pair 1's k/q.
            qk_mtile(2)   # k pair 0
            qk_mtile(0)   # q pair 0
            qk_mtile(3)   # k pair 1
            qk_mtile(1)   # q pair 1
            # v tiles are emitted inside unit 0's group loop below: v[2g]
            # and v[2g+1] land in group g, ahead of the A@V (skewed to g+1)
            # that first reads them — ACT starts ~30us earlier this way.

            # ---- attention: software-pipelined emission ----------------
            # Units are (pair, qt), qt-major so each 512-row block of the
            # output projection can be emitted as PE filler right after its
            # two units finish.  Within the global stream, the A@V matmuls
            # for group t are emitted AFTER the score matmuls of group t+1:
            # the PE is in-order, so this one-group skew keeps it from
            # stalling on the exp (ACT) results.
            GROUP = 2  # kv chunks per exp batch (PSUM tile = 2 banks)
            NGRP = KV_CHUNKS // GROUP

            pending_muls = []

            def flush_muls():
                while pending_muls:
                    pending_muls.pop(0)()

            def normalize_pair(o_acc_pair, pair, qt):
                # Stage both unnormalized accumulators to SBUF immediately so
                # the PSUM banks free for the next unit's A@V.  The division
                # chain uses only DVE + the idle sync/gpsimd DMA queues; the
                # final multiplies are deferred one unit so the in-order DVE
                # stream never waits on the broadcast DMA round-trip.
                for hx, po in ((0, 0), (1, D)):
                    ou = tmp.tile([VB, 512], f32, tag="ou", bufs=4,
                                  name=f"ou{pair}_{qt}_{hx}")
                    nc.vector.tensor_copy(out=ou[:],
                                          in_=o_acc_pair[hx][:VB])
                    # partition-spread the 512 denominators to [128, 4] via
                    # a DRAM hop so reciprocal runs 4 elems/lane, not 512
                    den_d = dscratch.tile([1, 512], f32, tag="dend",
                                          name=f"dend{pair}_{qt}_{hx}")
                    nc.sync.dma_start(den_d[:], ou[D:D + 1, :])
                    den_p = tmp.tile([P, 4], f32, tag="denp",
                                     name=f"denp{pair}_{qt}_{hx}")
                    nc.sync.dma_start(
                        den_p[:],
                        den_d[:].rearrange("o (j p) -> p (o j)", p=P))
                    rec_p = tmp.tile([P, 4], f32, tag="recp",
                                     name=f"recp{pair}_{qt}_{hx}")
                    nc.vector.reciprocal(out=rec_p[:], in_=den_p[:])
                    rec_d = dscratch.tile([1, 512], f32, tag="recd",
                                          name=f"recd{pair}_{qt}_{hx}")
                    nc.sync.dma_start(
                        rec_d[:].rearrange("o (j p) -> p (o j)", p=P),
                        rec_p[:])
                    bc_sb = tmp.tile([D, 512], f32, tag="bcsb", bufs=4,
                                     name=f"bcsb{pair}_{qt}_{hx}")
                    rec_bcast = bass.AP(
                        tensor=rec_d.tensor, offset=rec_d.offset,
                        ap=[[0, D]] + [list(p) for p in rec_d.ap[1:]])
                    nc.sync.dma_start(bc_sb[:], rec_bcast)

                    def mul(ou=ou, bc_sb=bc_sb, po=po, pair=pair, qt=qt):
                        nc.vector.tensor_mul(
                            out=oT_sb[po:po + D, pair,
                                      qt * 512:(qt + 1) * 512],
                            in0=ou[:D, :], in1=bc_sb[:])
                    pending_muls.append(mul)

            def proj_block(qt):
                # partial output projection for rows [qt*512, qt*512+512)
                for mt4 in range(4):
                    mt = qt * 4 + mt4
                    pp = ps_big.tile([P, 1024], f32, tag="sc",
                                     name=f"pp{mt}")
                    for nh in range(2):
                        for j in range(CG // P):
                            nc.tensor.matmul(
                                pp[:, nh * 512:nh * 512 + 512],
                                oT_sb[:, j, mt * P:(mt + 1) * P],
                                wp_sb[:, j, nh * 512:(nh + 1) * 512],
                                start=(j == 0), stop=(j == CG // P - 1))
                    ysb = tmp.tile([P, 1024], f16, tag="ysb",
                                   name=f"ysb{mt}")
                    if mt % 2 == 0:
                        nc.vector.tensor_copy(out=ysb[:], in_=pp[:])
                    else:
                        nc.scalar.copy(out=ysb[:], in_=pp[:])
                    eng = (nc.sync, nc.scalar, nc.gpsimd)[mt % 3]
                    eng.dma_start(yp.ap()[mt * P:(mt + 1) * P, :], ysb[:])

            units = [(pair, qt) for qt in range(QT) for pair in range(HG // 2)]
            o_accs_u = {}
            pending = None      # (u, g) whose A@V is not yet emitted

            def emit_av(u, g, exs):
                pair, qt = units[u]
                hA, hB = 2 * pair, 2 * pair + 1
                for i in range(GROUP):
                    r = g * GROUP + i
                    for hx, h in ((0, hA), (1, hB)):
                        nc.tensor.matmul(
                            o_accs_u[u][hx][:VB, :],
                            v_sb[:, r, h * VB:(h + 1) * VB],
                            exs[hx][:, i * 512:i * 512 + 512],
                            start=(r == 0), stop=(r == KV_CHUNKS - 1))
                if g == NGRP - 1:
                    normalize_pair(o_accs_u[u], pair, qt)
                    # emit the PREVIOUS unit's muls now (their broadcast
                    # DMAs completed long ago) — keeps DVE from stalling
                    while len(pending_muls) > 2:
                        pending_muls.pop(0)()
                    del o_accs_u[u]

            for u, (pair, qt) in enumerate(units):
                qs = slice(qt * 512, (qt + 1) * 512)
                o_accs_u[u] = [ps1.tile([P, 512], f32, tag="ps1",
                                        name=f"oacc{pair}_{qt}_{i}")
                               for i in range(2)]
                for g in range(NGRP):
                    if u == 0:
                        v_rtile(2 * g)
                        v_rtile(2 * g + 1)
                    scs = [ps_big.tile([P, 1024], f32, tag="sc",
                                       name=f"sc{pair}_{qt}_{g}_{i}")
                           for i in range(2)]
                    for i in range(GROUP):
                        r = g * GROUP + i
                        for hx, po in ((0, 0), (1, D)):
                            nc.tensor.matmul(
                                scs[hx][:, i * 512:i * 512 + 512],
                                kT_sb[po:po + D, pair, r * P:(r + 1) * P],
                                qT_sb[po:po + D, pair, qs],
                                start=True, stop=True)
                    exs = []
                    for hx in range(2):
                        ex = tmp.tile([P, 1024], bf16, tag="ex", bufs=6,
                                      name=f"ex{pair}_{qt}_{g}_{hx}")
                        nc.scalar.activation(
                            ex[:], scs[hx][:], AF.Exp, scale=SCALE)
                        exs.append(ex)
                    if pending is not None:
                        emit_av(*pending)
                    pending = (u, g, exs)
            emit_av(*pending)
            flush_muls()

            for qt in range(QT):
                proj_block(qt)

    nc.compile()
    return nc


def _host_prep(x, w_qkv, w_proj, b_proj):
    import ml_dtypes
    bf16 = ml_dtypes.bfloat16
    wqkvT = np.ascontiguousarray(w_qkv.T).astype(bf16)   # [C, 3C]
    wpT_full = np.ascontiguousarray(w_proj.T).astype(bf16)  # [C(in), C(out)]
    in_maps = []
    for c in range(NCORES):
        b, g = divmod(c, GROUPS)
        qcols = wqkvT[:, CG * g:CG * (g + 1)]
        kcols = wqkvT[:, C + CG * g:C + CG * (g + 1)]
        vcols = wqkvT[:, 2 * C + CG * g:2 * C + CG * (g + 1)]
        wqk = np.ascontiguousarray(np.concatenate([qcols, kcols], axis=1))
        wv = np.ascontiguousarray(vcols)
        wp = np.ascontiguousarray(wpT_full[CG * g:CG * (g + 1), :])
        xTv = np.ascontiguousarray(x[b].T).astype(bf16)
        in_maps.append({"xT": xTv, "wqkT": wqk, "wvT": wv, "wpT": wp})
    return in_maps


def run(inputs, trace=False, nc=None):
    """Build (or reuse) the program, run on 8 cores, return (y, results)."""
    global _CACHED_NC
    from concourse.bass_utils import run_bass_kernel_spmd
    if nc is None:
        if _CACHED_NC is None:
            _CACHED_NC = _build_nc()
        nc = _CACHED_NC
    in_maps = _host_prep(**inputs)
    res = run_bass_kernel_spmd(nc, in_maps, core_ids=list(range(NCORES)),
                               trace=trace)
    bias = np.asarray(inputs["b_proj"], np.float32)
    out = np.empty((B, N, C), np.float32)
    for b in range(B):
        acc = res.results[b * GROUPS]["yp"].astype(np.float32)
        for g in range(1, GROUPS):
            acc = acc + res.results[b * GROUPS + g]["yp"]
        out[b] = acc + bias
    return out, res


def kernel(x, w_qkv, w_proj, b_proj):
    out, _ = run({"x": np.asarray(x), "w_qkv": np.asarray(w_qkv),
                  "w_proj": np.asarray(w_proj), "b_proj": np.asarray(b_proj)})
    return out

